# revision 1
# baseline (speedup 1.0000x reference)
"""Trainium2 Bass kernel for nn_MultiHeadAttention_85925115723936.

Contract: kernel(**inputs) takes the FULL unsharded inputs from
setup_inputs() (x [16,1024,1024] f32, Wq/Wk/Wv [1024,64], Wp [1024,1024],
bp [1024]) and returns the FULL [16, 1024, 1024] float32 output.

Sharding: data-parallel over batch — 16 batches across 8 NeuronCores
(2 per core), zero cross-core communication.

All H=16 heads share one weight set (ModuleList([head]*H)), so the H-way
concat of the head output collapses into a folded projection:
  tile(head_out, H) @ Wp == head_out @ sum_h Wp[h*hs:(h+1)*hs, :].
Per batch the device computes (bf16 matmuls, f32 PSUM accumulation):
  [qT|kT] = [Wq/sqrt(hs) | Wk].T @ x.T      (one M=128 matmul group)
  vT      = Wv.T @ x.T
  scoresT[s,t] = sum_h kT[h,s] qT[h,t]       (h-contraction, K=64)
  expT = exp(scoresT) * causal_mask          (scores are O(1): the max-
                                              subtraction pass is unnecessary)
  [head_outT ; denom] = v_aug.T @ expT       (v augmented with a ones column
                                              so the softmax denominator falls
                                              out of the same matmul)
  out[t,:] = (ndT.T @ wp_aug)[t,:] / denom[t]   (bias folded into wp_aug)
Only the causal triangle of scoresT/expT/nd is computed.
"""

import numpy as np
import ml_dtypes

import concourse.mybir as mybir
import concourse.tile as tile
from concourse import bacc
from concourse.bass_utils import run_bass_kernel_spmd

BF_NP = ml_dtypes.bfloat16
BF = mybir.dt.bfloat16
F32 = mybir.dt.float32

B, T, D, H, HS = 16, 1024, 1024, 16, 64
NCORES = 8
NB = B // NCORES     # batches per core
NCH = 8              # 1024 / 128 chunks
OUT_DT = BF          # bf16 output halves the store traffic; rounding is ~0.2%


def _build_nc(n_reps: int = 1, n_cores: int = NCORES):
    nc = bacc.Bacc("TRN2", target_bir_lowering=False, debug=False, num_devices=n_cores)

    xT_d = nc.dram_tensor("xT", [NB, D, T], BF, kind="ExternalInput")
    wqk_d = nc.dram_tensor("wqk", [D, 128], BF, kind="ExternalInput")
    wv_d = nc.dram_tensor("wv", [D, HS], BF, kind="ExternalInput")
    wp_d = nc.dram_tensor("wp_aug", [HS + 1, D], BF, kind="ExternalInput")
    mask_d = nc.dram_tensor("mask", [128, 128], BF, kind="ExternalInput")
    idb_d = nc.dram_tensor("ident_bf", [128, 128], BF, kind="ExternalInput")
    unit_d = nc.dram_tensor("unit65", [HS + 1, 1], BF, kind="ExternalInput")
    out_d = nc.dram_tensor("out", [NB, T, D], OUT_DT, kind="ExternalOutput")

    with tile.TileContext(nc) as tc:
        with (
            tc.tile_pool(name="const", bufs=1) as cpool,
            tc.tile_pool(name="sb", bufs=2) as sb,
            tc.tile_pool(name="psA", bufs=2, space="PSUM") as psA,
            tc.tile_pool(name="psB", bufs=4, space="PSUM") as psB,
        ):
            # constants / weights on the ACT HWDGE ring so they don't block
            # the first xT load on the SP ring
            wqk_sb = cpool.tile([128, NCH, 128], BF, tag="wqk")
            wqk_r = wqk_d.ap().rearrange("(c p) m -> p c m", p=128)
            nc.scalar.dma_start(out=wqk_sb[:, 0:1, :], in_=wqk_r[:, 0:1, :])
            nc.scalar.dma_start(out=wqk_sb[:, 1:NCH, :], in_=wqk_r[:, 1:NCH, :])
            wv_sb = cpool.tile([128, NCH, HS], BF, tag="wv")
            nc.scalar.dma_start(
                out=wv_sb[:], in_=wv_d.ap().rearrange("(c p) m -> p c m", p=128)
            )
            wp_sb = cpool.tile([HS + 1, D], BF, tag="wp")
            nc.scalar.dma_start(out=wp_sb[:], in_=wp_d.ap())
            mask_sb = cpool.tile([128, 128], BF, tag="mask")
            nc.scalar.dma_start(out=mask_sb[:], in_=mask_d.ap())
            idb_sb = cpool.tile([128, 128], BF, tag="idb")
            nc.scalar.dma_start(out=idb_sb[:], in_=idb_d.ap())
            unit_sb = cpool.tile([HS + 1, 1], BF, tag="unit")
            nc.scalar.dma_start(out=unit_sb[:], in_=unit_d.ap())

            def load_batch(b):
                # x^T in graded pieces so the first matmuls start sooner; all
                # loads are emitted before any store so a store's sem wait
                # never delays the next batch's prefetch in the SP stream
                xr = xT_d.ap()[b].rearrange("(c p) t -> p c t", p=128)
                # first chunk split by t-half so the very first matmul starts
                # as soon as 128KB has landed
                xt0a = sb.tile([128, 512], BF, tag="xt0a")
                nc.sync.dma_start(out=xt0a[:], in_=xr[:, 0, 0:512])
                xt0b = sb.tile([128, 512], BF, tag="xt0b")
                nc.sync.dma_start(out=xt0b[:], in_=xr[:, 0, 512:T])
                xq = []
                for q, (c0, c1) in enumerate([(1, 2), (2, 4), (4, 8)]):
                    t_ = sb.tile([128, c1 - c0, T], BF, tag=f"xt{q}")
                    nc.sync.dma_start(out=t_[:], in_=xr[:, c0:c1, :])
                    xq.append((c0, c1, t_))
                return (xt0a, xt0b, xq)

            def batch_body(b, xload):
                xt0a, xt0b, xq = xload

                def xt_slice(c, h):
                    if c == 0:
                        return (xt0a if h == 0 else xt0b)[:]
                    for c0, c1, t_ in xq:
                        if c0 <= c < c1:
                            return t_[:, c - c0, h * 512 : (h + 1) * 512]
                    raise AssertionError

                # q^T (psum rows 0..63) and k^T (rows 64..127)
                qk_ps = psA.tile([128, T], F32, tag="big")
                for c in range(NCH):
                    for h in range(2):
                        nc.tensor.matmul(
                            qk_ps[:, h * 512 : (h + 1) * 512],
                            wqk_sb[:, c, :],
                            xt_slice(c, h),
                            start=(c == 0),
                            stop=(c == NCH - 1),
                        )
                qT = sb.tile([HS, T], BF, tag="qT")
                kT = sb.tile([HS, T], BF, tag="kT")
                nc.vector.tensor_copy(qT[:], qk_ps[0:HS, :])
                nc.scalar.copy(kT[:], qk_ps[HS:128, :])

                # v^T
                v_ps = psA.tile([HS, T], F32, tag="big")
                for c in range(NCH):
                    for h in range(2):
                        nc.tensor.matmul(
                            v_ps[:, h * 512 : (h + 1) * 512],
                            wv_sb[:, c, :],
                            xt_slice(c, h),
                            start=(c == 0),
                            stop=(c == NCH - 1),
                        )
                vT = sb.tile([HS, T], BF, tag="vT")
                nc.vector.tensor_copy(vT[:], v_ps[:])

                # scoresT per s-chunk (exp over the causal triangle, diagonal
                # masked on GPSIMD), v_aug transposes interleaved as PE filler.
                # Two stage groups so nd/out work for chunks 0-3 overlaps the
                # scores/exp of chunks 4-7.
                v_aug = sb.tile([128, NCH, HS + 1], BF, tag="vaug")
                nc.gpsimd.memset(v_aug[:, :, HS], 1.0)
                attnT = sb.tile([128, NCH, T], BF, tag="attnT")
                ndT = sb.tile([HS + 1, NCH, 128], BF, tag="ndT")
                recip = sb.tile([128, NCH], F32, tag="recip")
                out_sb = sb.tile([128, NCH, D], OUT_DT, tag="out")

                def scores_chunk(i):
                    t0 = 0 if i < 4 else 512
                    sc_ps = psA.tile([128, T], F32, tag="big")
                    for h in range(t0 // 512, 2):
                        nc.tensor.matmul(
                            sc_ps[:, h * 512 : (h + 1) * 512],
                            kT[:, i * 128 : (i + 1) * 128],
                            qT[:, h * 512 : (h + 1) * 512],
                            start=True,
                            stop=True,
                        )
                    tp_ps = psB.tile([128, HS], BF, tag="small")
                    nc.tensor.transpose(
                        tp_ps[:], vT[:, i * 128 : (i + 1) * 128], idb_sb[0:HS, 0:HS]
                    )
                    nc.vector.tensor_copy(v_aug[:, i, 0:HS], tp_ps[:])
                    te = 128 * i  # exp only needed for t >= s-chunk start
                    nc.scalar.activation(
                        attnT[:, i, te:T],
                        sc_ps[:, te:T],
                        mybir.ActivationFunctionType.Exp,
                    )
                    nc.gpsimd.tensor_mul(
                        attnT[:, i, i * 128 : (i + 1) * 128],
                        attnT[:, i, i * 128 : (i + 1) * 128],
                        mask_sb[:],
                    )

                def nd_group(j0, j1):
                    nd_ps = psB.tile([HS + 1, j1 - j0, 128], F32, tag="small")
                    for j in range(j0, j1):
                        for i in range(j + 1):
                            nc.tensor.matmul(
                                nd_ps[:, j - j0, :],
                                v_aug[:, i, :],
                                attnT[:, i, j * 128 : (j + 1) * 128],
                                start=(i == 0),
                                stop=(i == j),
                            )
                    nc.vector.tensor_copy(ndT[:, j0:j1, :], nd_ps[:])

                def out_chunk(j):
                    o_ps = psA.tile([128, D], F32, tag="big")
                    dT_ps = psB.tile([128, 1], F32, tag="small")
                    for h in range(2):
                        nc.tensor.matmul(
                            o_ps[:, h * 512 : (h + 1) * 512],
                            ndT[:, j, :],
                            wp_sb[:, h * 512 : (h + 1) * 512],
                            start=True,
                            stop=True,
                        )
                    # denominator column via a unit-vector matmul on the
                    # already-loaded ndT weights
                    nc.tensor.matmul(
                        dT_ps[:], ndT[:, j, :], unit_sb[:], start=True, stop=True
                    )
                    nc.vector.reciprocal(recip[:, j : j + 1], dT_ps[:])
                    if j == NCH - 1:
                        # final chunk: split the norm across both engines and
                        # store two halves to shorten the kernel tail
                        nc.scalar.activation(
                            out_sb[:, j, 0:512],
                            o_ps[:, 0:512],
                            mybir.ActivationFunctionType.Copy,
                            scale=recip[:, j : j + 1],
                        )
                        nc.vector.tensor_scalar_mul(
                            out_sb[:, j, 512:D], o_ps[:, 512:D], recip[:, j : j + 1]
                        )
                        orr = out_d.ap()[b].rearrange("(c p) e -> p c e", p=128)
                        nc.sync.dma_start(
                            out=orr[:, j : j + 1, 0:512],
                            in_=out_sb[:, j : j + 1, 0:512],
                        )
                        nc.sync.dma_start(
                            out=orr[:, j : j + 1, 512:D],
                            in_=out_sb[:, j : j + 1, 512:D],
                        )
                        return
                    if j % 2 == 1:
                        nc.scalar.activation(
                            out_sb[:, j, :],
                            o_ps[:],
                            mybir.ActivationFunctionType.Copy,
                            scale=recip[:, j : j + 1],
                        )
                    else:
                        nc.vector.tensor_scalar_mul(
                            out_sb[:, j, :], o_ps[:], recip[:, j : j + 1]
                        )
                    nc.sync.dma_start(
                        out=out_d.ap()[b].rearrange("(c p) e -> p c e", p=128)[
                            :, j : j + 1, :
                        ],
                        in_=out_sb[:, j : j + 1, :],
                    )

                for i in range(4):
                    scores_chunk(i)
                nd_group(0, 4)
                for i in range(4, NCH):
                    scores_chunk(i)
                for j in range(4):
                    out_chunk(j)
                nd_group(4, 6)
                out_chunk(4)
                out_chunk(5)
                nd_group(6, NCH)
                out_chunk(6)
                out_chunk(7)

            def all_batches(_=None):
                xqs = [load_batch(b) for b in range(NB)]
                for b in range(NB):
                    batch_body(b, xqs[b])

            if n_reps == 1:
                all_batches()
            else:
                with tc.For_i(0, n_reps, 1):
                    all_batches()

    nc.compile()
    return nc


def _prep_inputs(x, Wq, Wk, Wv, Wp, bp):
    x = np.asarray(x, np.float32)
    Wq = np.asarray(Wq, np.float32)
    Wk = np.asarray(Wk, np.float32)
    Wv = np.asarray(Wv, np.float32)
    Wp = np.asarray(Wp, np.float32)
    bp = np.asarray(bp, np.float32)

    # fold the H-way tile-concat into Wp, the 1/sqrt(hs) scale into Wq
    Wp_eff = Wp.reshape(H, HS, D).sum(0)
    wp_aug = np.concatenate([Wp_eff, bp[None, :]], 0).astype(BF_NP)
    wqk = np.concatenate([Wq * np.float32(1.0 / np.sqrt(HS)), Wk], 1).astype(BF_NP)
    wv = Wv.astype(BF_NP)
    xT = np.ascontiguousarray(x.transpose(0, 2, 1)).astype(BF_NP)

    mask = np.triu(np.ones((128, 128), np.float32)).astype(BF_NP)
    ident_bf = np.eye(128, dtype=np.float32).astype(BF_NP)
    unit65 = np.zeros((65, 1), np.float32)
    unit65[64, 0] = 1.0
    unit65 = unit65.astype(BF_NP)

    in_maps = []
    for c in range(NCORES):
        in_maps.append(
            {
                "xT": np.ascontiguousarray(xT[c * NB : (c + 1) * NB]),
                "wqk": wqk,
                "wv": wv,
                "wp_aug": wp_aug,
                "mask": mask,
                "ident_bf": ident_bf,
                "unit65": unit65,
            }
        )
    return in_maps


_NC_CACHE = {}


def kernel(x, Wq, Wk, Wv, Wp, bp):
    in_maps = _prep_inputs(x, Wq, Wk, Wv, Wp, bp)
    if "nc" not in _NC_CACHE:
        _NC_CACHE["nc"] = _build_nc(n_reps=1)
    nc = _NC_CACHE["nc"]
    last_err = None
    for _ in range(3):  # retry: the axon transport occasionally hiccups
        try:
            res = run_bass_kernel_spmd(nc, in_maps, core_ids=list(range(NCORES)))
            out = np.concatenate([np.asarray(r["out"]) for r in res.results], 0)
            return np.ascontiguousarray(out.astype(np.float32))
        except Exception as e:  # noqa: BLE001
            last_err = e
    raise last_err

